# revision 16
# baseline (speedup 1.0000x reference)
"""BiLSTM model kernel for 8 Trainium2 NeuronCores.

Model (matches reference): e = emb[x]; h_f = LSTM_fwd(e)[-1]; h_b = LSTM_bwd(e)[-1];
out = sigmoid(concat(h_f, h_b) @ fc_w.T + fc_b).

Sharding: 8 cores = 4 batch shards (64 rows each) x 2 directions. Every core runs
the identical SPMD program: a 512-step LSTM scan for one direction over its
batch shard. The backward direction is realized by feeding the time-reversed
token sequence. Weights are pre-packed on host into transposed layouts; biases
ride as an extra "ones" row of the embedding matrix (contraction dim K = 101).

Per-step layout: hidden dim on partitions 0:64, gates in COLUMN blocks (batch
on free dim) sharing one PSUM bank, so every elementwise op stays on the same
partitions (DVE/ACT are lane-locked and cannot pair data across partitions):
  PSUM P4 [64, 4B] = [a_f | a_i | a_g | a_o]  (e-proj + h-proj accumulated)
  X3 = sigmoid(f,i,o blocks) -> [sf | si | so];  Y2 [64, 2B] = [tanh(g) | c]
  PR = [sf*c | si*tg];  c' = PR[:,0:B] + PR[:,B:2B];  h = so * tanh(c')
The embedding lookup runs on-device (indirect-DMA row gathers + PE transpose),
pipelined ahead of the scan and off the recurrent critical path.
"""

import sys

sys.path.insert(0, "/opt/trn_rl_repo")

import numpy as np

import concourse.bacc as bacc
import concourse.bass as bass
import concourse.mybir as mybir
import concourse.tile as tile
from concourse.bass_utils import run_bass_kernel_spmd
from concourse.masks import make_identity

F32 = mybir.dt.float32
AF = mybir.ActivationFunctionType
ALU = mybir.AluOpType

V, E, HID, B, S = 50000, 100, 64, 256, 512
N_CORES = 8
BC = B // 4  # 64 batch rows per core; cores 0-3 forward, 4-7 backward
K = E + 1  # contraction dim: embedding dims + ones row (bias)

_built = {}


def _build(s_len=S, bc=BC, repeats=1, gather=True):
    """Build + compile the single SPMD program (one LSTM direction scan).

    gather=True does the embedding lookup on-device: indirect-DMA row
    gathers from the replicated emb table (128 rows per call), PE-transpose
    to [E, tokens] layout, copy into the resident eT SBUF tile. All of it is
    off the recurrent chain and overlaps the scan.

    repeats > 1 runs the whole scan that many times (state reset in between;
    output comes from the last repeat) — used to measure pure scan time as a
    slope, free of dispatch overhead."""
    key = (s_len, bc, repeats, gather)
    if key in _built:
        return _built[key]

    nc = bacc.Bacc("TRN2", target_bir_lowering=False, debug=False, num_devices=N_CORES)

    n_tok = s_len * bc
    n_chunks = (n_tok + 127) // 128
    if gather:
        emb_d = nc.dram_tensor("emb", [V, E], F32, kind="ExternalInput")
        idx_d = nc.dram_tensor("idx", [128, n_chunks], mybir.dt.int32,
                               kind="ExternalInput")
        ones_d = nc.dram_tensor("ones_row", [1, n_tok], F32, kind="ExternalInput")
    else:
        eT = nc.dram_tensor("eT", [K, n_tok], F32, kind="ExternalInput")
    # gate column order: i, f, o, g
    w_all = nc.dram_tensor("w_all", [K, 256], F32, kind="ExternalInput")
    u_all = nc.dram_tensor("u_all", [HID, 256], F32, kind="ExternalInput")
    y = nc.dram_tensor("y", [HID, bc], F32, kind="ExternalOutput")

    with tile.TileContext(nc) as tc:
        with (
            tc.tile_pool(name="const", bufs=1) as cpool,
            tc.tile_pool(name="state", bufs=1) as spool,
            tc.tile_pool(name="step", bufs=4) as pool,
            tc.tile_pool(name="gath", bufs=10) as gpool,
            tc.tile_pool(name="psum", bufs=4, space="PSUM") as ppool,
            tc.tile_pool(name="psumT", bufs=3, space="PSUM") as ptpool,
        ):
            eT_sb = cpool.tile([K, n_tok], F32)
            if gather:
                idx_sb = cpool.tile([128, n_chunks], mybir.dt.int32)
                nc.sync.dma_start(out=idx_sb[:], in_=idx_d[:])
                nc.sync.dma_start(out=eT_sb[E : E + 1, :], in_=ones_d[:])
                ident = cpool.tile([128, 128], F32)
                make_identity(nc, ident[:])
            else:
                nc.sync.dma_start(out=eT_sb[:], in_=eT[:])
            w_sb = cpool.tile([K, 256], F32)
            nc.sync.dma_start(out=w_sb[:], in_=w_all[:])
            u_sb = cpool.tile([HID, 256], F32)
            nc.sync.dma_start(out=u_sb[:], in_=u_all[:])

            def gather_chunk(c):
                """Gather 128 embedding rows for chunk c and transpose them
                into eT_sb[0:E, c*128:(c+1)*128]."""
                R = gpool.tile([128, E], F32, tag="R")
                nc.gpsimd.indirect_dma_start(
                    out=R[:],
                    out_offset=None,
                    in_=emb_d[:],
                    in_offset=bass.IndirectOffsetOnAxis(
                        ap=idx_sb[:, c : c + 1], axis=0
                    ),
                )
                pT = ptpool.tile([E, 128], F32, tag="pT")
                nc.tensor.transpose(out=pT[:], in_=R[:], identity=ident[:])
                # alternate the copy engine to split the overhead
                eng = nc.scalar if c % 2 == 0 else nc.vector
                if eng is nc.scalar:
                    eng.copy(eT_sb[0:E, c * 128 : (c + 1) * 128], pT[:])
                else:
                    eng.tensor_copy(
                        out=eT_sb[0:E, c * 128 : (c + 1) * 128], in_=pT[:]
                    )

            # Y2[:, 0:bc] = tanh(g) slot (written each step), Y2[:, bc:2bc] = c
            Y2 = spool.tile([HID, 2 * bc], F32)

            # All four gate blocks share ONE PSUM bank (4*bc*4B = 1KB < 2KB):
            # the first matmul's start=True pending-zeroes the whole bank, so
            # later e-proj blocks overwrite-on-first-write and h-projs
            # accumulate. Block order [f | i | g | o]: sigmoid(f,i) is needed
            # first (sf*c can start while tanh(g) still runs on ACT).
            def step(t, h_prev):
                P4 = ppool.tile([HID, 4 * bc], F32, tag="P4")
                ecol = eT_sb[:, t * bc : (t + 1) * bc]
                first = h_prev is None

                # e-projections (off the recurrent chain)
                for q in range(4):
                    wq = [1, 0, 3, 2][q]  # block f,i,g,o <- w_all cols i,f,o,g
                    nc.tensor.matmul(
                        P4[:, q * bc : (q + 1) * bc],
                        lhsT=w_sb[:, wq * 64 : (wq + 1) * 64],
                        rhs=ecol,
                        start=(q == 0),
                        stop=first and q == 3,
                    )
                # h-projections (on the chain); f,i first for early sigmoid
                if not first:
                    for q in range(4):
                        wq = [1, 0, 3, 2][q]
                        nc.tensor.matmul(
                            P4[:, q * bc : (q + 1) * bc],
                            lhsT=u_sb[:, wq * 64 : (wq + 1) * 64],
                            rhs=h_prev[:],
                            start=False,
                            stop=q == 3,
                        )

                X3 = pool.tile([HID, 3 * bc], F32, tag="X3")  # [sf | si | so]
                nc.scalar.activation(X3[:, 0 : 2 * bc], P4[:, 0 : 2 * bc], AF.Sigmoid)
                nc.scalar.activation(Y2[:, 0:bc], P4[:, 2 * bc : 3 * bc], AF.Tanh)
                nc.scalar.activation(
                    X3[:, 2 * bc : 3 * bc], P4[:, 3 * bc : 4 * bc], AF.Sigmoid
                )

                PR = pool.tile([HID, 2 * bc], F32, tag="PR")
                nc.vector.tensor_tensor(  # sf * c (early: only needs A_fi)
                    out=PR[:, 0:bc], in0=X3[:, 0:bc], in1=Y2[:, bc : 2 * bc],
                    op=ALU.mult,
                )
                nc.vector.tensor_tensor(  # si * tanh(g)
                    out=PR[:, bc : 2 * bc], in0=X3[:, bc : 2 * bc],
                    in1=Y2[:, 0:bc], op=ALU.mult,
                )
                nc.vector.tensor_tensor(  # c' into the c slot
                    out=Y2[:, bc : 2 * bc], in0=PR[:, 0:bc],
                    in1=PR[:, bc : 2 * bc], op=ALU.add,
                )
                TC = pool.tile([HID, bc], F32, tag="TC")
                nc.scalar.activation(TC[:], Y2[:, bc : 2 * bc], AF.Tanh)
                Hn = pool.tile([HID, bc], F32, tag="H")
                nc.vector.tensor_tensor(
                    out=Hn[:], in0=X3[:, 2 * bc : 3 * bc], in1=TC[:], op=ALU.mult
                )
                return Hn

            PF = 4  # chunks of gather prefetch ahead of the scan
            for _rep in range(repeats):
                nc.vector.memset(Y2[:], 0.0)
                if gather and _rep == 0:
                    for c in range(min(PF, n_chunks)):
                        gather_chunk(c)
                h_prev = None
                for t in range(s_len):
                    if gather and _rep == 0 and t % 2 == 0:
                        c = t // 2 + PF
                        if c < n_chunks:
                            gather_chunk(c)
                    h_prev = step(t, h_prev)

            nc.sync.dma_start(out=y[:], in_=h_prev[:])

    nc.compile()
    _built[key] = nc
    return nc


def _pack_weights(W_ih, W_hh, b_ih, b_hh):
    """Host-side packing for one direction: w_all [K, 256] (cols = gates
    i|f|o|g, bias in row E), u_all [64, 256]."""
    b = (b_ih + b_hh).astype(np.float32)
    order = [0, 1, 3, 2]  # i, f, o, g (reference gate order is i,f,g,o)
    w = np.concatenate([W_ih[q * HID : (q + 1) * HID] for q in order], axis=0).T
    bias = np.concatenate([b[q * HID : (q + 1) * HID] for q in order])[None, :]
    w_all = np.concatenate([w, bias], axis=0)
    u_all = np.concatenate([W_hh[q * HID : (q + 1) * HID] for q in order], axis=0).T
    return (
        np.ascontiguousarray(w_all, dtype=np.float32),
        np.ascontiguousarray(u_all, dtype=np.float32),
    )


def _prepare_in_maps(inputs, s_len=S, bc=BC, gather=True):
    x = np.asarray(inputs["x"])
    emb = np.asarray(inputs["emb"], dtype=np.float32)
    emb = np.ascontiguousarray(emb)
    pk_f = _pack_weights(
        np.asarray(inputs["W_ih_f"], np.float32), np.asarray(inputs["W_hh_f"], np.float32),
        np.asarray(inputs["b_ih_f"], np.float32), np.asarray(inputs["b_hh_f"], np.float32),
    )
    pk_b = _pack_weights(
        np.asarray(inputs["W_ih_b"], np.float32), np.asarray(inputs["W_hh_b"], np.float32),
        np.asarray(inputs["b_ih_b"], np.float32), np.asarray(inputs["b_hh_b"], np.float32),
    )

    batch = x.shape[0]
    n_shards = batch // bc
    n_tok = s_len * bc
    ones = np.ones((1, n_tok), dtype=np.float32)

    in_maps = []
    for core in range(N_CORES):
        fwd = core < n_shards
        shard = core % n_shards
        xs = x[shard * bc : (shard + 1) * bc, :s_len]  # [bc, s]
        if not fwd:
            xs = xs[:, ::-1]
        w_all, u_all = pk_f if fwd else pk_b
        m = {"w_all": w_all, "u_all": u_all}
        if gather:
            # token j = t*bc + b -> emb row x[b, t]; idx[p, c] covers j = c*128+p
            tok = np.ascontiguousarray(xs.T.reshape(-1).astype(np.int32))  # [n_tok]
            m["idx"] = np.ascontiguousarray(tok.reshape(-1, 128).T)  # [128, n_chunks]
            m["emb"] = emb
            m["ones_row"] = ones
        else:
            # eT column j = t*bc + b holds emb[x[b, t]] (+ ones row for bias)
            e = emb[xs.T.reshape(-1)]  # [s*bc, E]
            eT_core = np.concatenate([np.ascontiguousarray(e.T), ones], axis=0)
            m["eT"] = np.ascontiguousarray(eT_core, dtype=np.float32)
        in_maps.append(m)
    return in_maps


def _postprocess(results, inputs, bc=BC):
    fc_w = np.asarray(inputs["fc_w"], dtype=np.float32)
    fc_b = np.asarray(inputs["fc_b"], dtype=np.float32)
    n_shards = np.asarray(inputs["x"]).shape[0] // bc
    h_f = np.concatenate([results[c]["y"].T for c in range(n_shards)], axis=0)
    h_b = np.concatenate(
        [results[n_shards + c]["y"].T for c in range(n_shards)], axis=0
    )
    h_cat = np.concatenate([h_f, h_b], axis=1)  # [B, 2H]
    out = 1.0 / (1.0 + np.exp(-(h_cat @ fc_w.T + fc_b)))
    return out.astype(np.float32)


def kernel(x, emb, W_ih_f, W_hh_f, b_ih_f, b_hh_f, W_ih_b, W_hh_b, b_ih_b, b_hh_b,
           fc_w, fc_b, s_len=S, bc=BC, gather=True):
    inputs = dict(
        x=x, emb=emb, W_ih_f=W_ih_f, W_hh_f=W_hh_f, b_ih_f=b_ih_f, b_hh_f=b_hh_f,
        W_ih_b=W_ih_b, W_hh_b=W_hh_b, b_ih_b=b_ih_b, b_hh_b=b_hh_b,
        fc_w=fc_w, fc_b=fc_b,
    )
    nc = _build(s_len, bc, gather=gather)
    in_maps = _prepare_in_maps(inputs, s_len, bc, gather=gather)
    res = run_bass_kernel_spmd(nc, in_maps, list(range(N_CORES)))
    return _postprocess(res.results, inputs, bc)
